# revision 17
# baseline (speedup 1.0000x reference)
"""Trainium2 Bass kernel for nn_CentralityEncoding (8 NeuronCores, SPMD).

Math (reference):
    out = x + z_in[min(in_deg,511)] + z_out[min(out_deg,511)]
        + sigmoid(cent @ W_cent + b_cent) + sigmoid(vor @ W_vor + b_vor)
        + segment_sum(edge_attr @ W_sum + b_sum, src)

Reformulations:
  * segment_sum(edge_attr @ W_sum + b_sum, src)
        = segment_sum(edge_attr, src) @ W_sum + out_deg_raw[:,None] * b_sum
    (cuts the big matmul from E=1.6M rows to N=50k rows)
  * z_in[deg] = onehot(deg) @ z_in accumulated into the same PSUM tile.
  * sigmoid(cent*W + 0) = one ACT op with per-partition scale operand.
  * x lands in the same PSUM via an identity matmul (lhsT=I).

Sharding: nodes are assigned to (core, 64-node sub-chunk) slots by an
LPT bin-packing over out-degree so every sub-chunk's edge count fits
t_cap*128 with ~zero padding; each edge is routed on host to its src
node's sub-chunk.  The device program is static and identical across
cores - no collectives.  Edge features ship as fp8 e3m4 (1B/value,
~1.3% rms) and are consumed directly by the PE (f32 PSUM accumulate);
everything else is bf16.

Per-sub-chunk segment sum: for each 128-edge tile build a selection
matrix sel[e,q] = (src_rel[e] == q) with one DVE is_equal per 128-node
group, laid out q-major (both operands stream dense, innermost step 1,
so DVE runs in 2x mode); then matmul lhsT=edge_tile [128e,128d],
rhs=sel column slice [128e,NW] accumulating agg[d,q] in PSUM.
"""

import numpy as np
import ml_dtypes
import heapq

import bass_rust
import concourse.bass as bass
import concourse.mybir as mybir
import concourse.tile as tile
from concourse.bass_utils import run_bass_kernel_spmd
from concourse.vector_clock import ScopedClock

# ----------------------------------------------------------------------------
# Problem constants (hardcoded per the harness contract).
N_NODES = 50000
N_EDGES = 1600000
NODE_DIM = 256
EDGE_DIM = 128
MAX_DEG = 512
N_CORES = 8
P = 128
NPC = N_NODES // N_CORES       # 6250 nodes per core
NPAIR = 49                     # groups of 128 nodes per core
NPAD = NPAIR * P               # 6272 padded node slots per core
F32 = mybir.dt.float32
BF16 = mybir.dt.bfloat16
FP8 = mybir.dt.float8e3        # e3m4

NW = 64                        # nodes per sub-chunk (sel width)
SPG = P // NW                  # sub-chunks per 128-node group
CB = NPAD // NW                # sub-chunks (bins) per core

AB = 8                         # groups per a-load / out-store block
XB = 8                         # groups per x-load block


# ----------------------------------------------------------------------------
# Workarounds for this container's walrus build, which rejects any
# instruction carrying more than ONE semaphore wait ("Too many sync wait
# commands", CoreV3GenImpl setupSyncWait).
_orig_commit = tile.TileContext._commit_instruction


def _commit_split_waits(self, inst, lazy_reg_writes=True):
    si = getattr(inst, "sync_info", None)
    if si is not None and si.on_wait and len(si.on_wait) > 1:
        waits = list(si.on_wait)
        for w in waits[:-1]:
            nop = mybir.InstNoOp(
                name=self.nc.get_next_instruction_name(),
                sync_info=mybir.SyncInfo(on_wait=[w], on_update=[]),
                bass_nofuse=True,
                engine=inst.engine,
            )
            _orig_commit(self, nop, lazy_reg_writes)
        inst.sync_info = mybir.SyncInfo(
            on_wait=[waits[-1]], on_update=list(si.on_update)
        )
    return _orig_commit(self, inst, lazy_reg_writes)


tile.TileContext._commit_instruction = _commit_split_waits


def _patched_drain_and_barrier(self, tick_clock, wait_clock):
    nc = self.nc
    collector = nc.sync.nop(nofuse=True)
    wait_clock.add_sem_waits(
        collector.ins, ScopedClock({None: tick_clock.global_clock})
    )
    si = collector.ins.sync_info
    waits = list(si.on_wait) if si is not None else []
    if waits:
        collector.ins.sync_info = bass_rust.SyncInfo(
            on_wait=[waits[0]], on_update=[]
        )
        for w in waits[1:]:
            nop = nc.sync.nop(nofuse=True)
            nop.ins.sync_info = bass_rust.SyncInfo(on_wait=[w], on_update=[])
    nc.sync.drain()
    nc.all_engine_barrier()
    assert self.sems is not None
    popped = nc._tile_sem_poison_stack.pop()
    assert popped is self._sem_poison
    nc.clear_and_free_semaphores(list(self.sems.allocated().values()))
    nc.all_engine_barrier()


tile.TileContext._drain_and_barrier = _patched_drain_and_barrier


# ----------------------------------------------------------------------------
def build_program(t_cap: int, n_groups: int, has_bsum: bool, has_bcent: bool,
                  has_bvor: bool) -> bass.Bass:
    TPG = SPG * t_cap          # 128-edge tiles per 128-node group
    GW = TPG * P               # a columns per group (fp8 bytes)
    nc = bass.Bass()

    a_d = nc.declare_dram_parameter("a", [NPAIR * P, GW], FP8, isOutput=False)
    # srel with each value repeated 16x: dense 16-elem runs keep the DVE
    # is_equal in 2x mode while emitting sel in tile-major layout
    srel_d = nc.declare_dram_parameter("srel", [P, CB * t_cap * 8], BF16, isOutput=False)
    x_d = nc.declare_dram_parameter("x", [NPAD, NODE_DIM], FP8, isOutput=False)
    idegr_d = nc.declare_dram_parameter("idegrow", [n_groups, NPAD], F32, isOutput=False)
    odegr_d = nc.declare_dram_parameter("odegrow", [n_groups, NPAD], F32, isOutput=False)
    cent_d = nc.declare_dram_parameter("cent", [P, NPAIR], F32, isOutput=False)
    vor_d = nc.declare_dram_parameter("vor", [P, NPAIR], F32, isOutput=False)
    zin_d = nc.declare_dram_parameter("z_in", [n_groups * P, NODE_DIM], BF16, isOutput=False)
    zout_d = nc.declare_dram_parameter("z_out", [n_groups * P, NODE_DIM], BF16, isOutput=False)
    wsum_d = nc.declare_dram_parameter("W_sum", [EDGE_DIM, NODE_DIM], BF16, isOutput=False)
    wcent_d = nc.declare_dram_parameter("W_cent", [1, NODE_DIM], F32, isOutput=False)
    wvor_d = nc.declare_dram_parameter("W_vor", [1, NODE_DIM], F32, isOutput=False)
    if has_bsum:
        odegraw_d = nc.declare_dram_parameter("odegraw", [P, NPAIR], F32, isOutput=False)
        bsum_d = nc.declare_dram_parameter("b_sum", [1, NODE_DIM], F32, isOutput=False)
    if has_bcent:
        bcent_d = nc.declare_dram_parameter("b_cent", [1, NODE_DIM], F32, isOutput=False)
    if has_bvor:
        bvor_d = nc.declare_dram_parameter("b_vor", [1, NODE_DIM], F32, isOutput=False)
    out_d = nc.declare_dram_parameter("out", [NPAD, NODE_DIM], BF16, isOutput=True)

    sig = mybir.ActivationFunctionType.Sigmoid
    cpy = mybir.ActivationFunctionType.Copy
    NAB = (NPAIR + AB - 1) // AB
    NXB = (NPAIR + XB - 1) // XB

    with tile.TileContext(nc) as tc:
        with (
            tc.tile_pool(name="const", bufs=1) as const,
            tc.tile_pool(name="apool", bufs=2) as apool,
            tc.tile_pool(name="rpool", bufs=2) as rpool,
            tc.tile_pool(name="xpool", bufs=3) as xpool,
            tc.tile_pool(name="spool", bufs=4) as spool,
            tc.tile_pool(name="aggp", bufs=8) as aggp,
            tc.tile_pool(name="tp", bufs=8) as tp,
            tc.tile_pool(name="opool", bufs=3) as opool,
            tc.tile_pool(name="psp", bufs=4, space="PSUM") as psp,
            tc.tile_pool(name="prp", bufs=3, space="PSUM") as prp,
        ):
            # --- one-time constants -------------------------------------
            # iota_q[p, tt*64 + r*16 + j] = 16r + j  (value q = c%64, t-major)
            iota_q = const.tile([P, TPG * NW], BF16, tag="iota_q")
            nc.gpsimd.iota(iota_q[:], pattern=[[0, TPG], [8, 8], [1, 8]],
                           base=0, channel_multiplier=0,
                           allow_small_or_imprecise_dtypes=True)
            # partition-index constant over the node axis (deg onehots)
            iotap_b = const.tile([P, NPAD], BF16, tag="iotap_b")
            nc.gpsimd.iota(iotap_b[:], pattern=[[0, NPAD]], base=0,
                           channel_multiplier=1,
                           allow_small_or_imprecise_dtypes=True)
            # identity (for the x pass-through matmul)
            iota_j = const.tile([P, P], BF16, tag="iota_j")
            nc.gpsimd.iota(iota_j[:], pattern=[[1, P]], base=0,
                           channel_multiplier=0,
                           allow_small_or_imprecise_dtypes=True)
            ident = const.tile([P, P], BF16, tag="ident")
            nc.vector.tensor_tensor(out=ident[:], in0=iotap_b[:, :P],
                                    in1=iota_j[:],
                                    op=mybir.AluOpType.is_equal)

            wsum_b = const.tile([EDGE_DIM, NODE_DIM], BF16, tag="wsum_b")
            nc.gpsimd.dma_start(out=wsum_b[:], in_=wsum_d[:])

            zin_sb, zout_sb, ideg_oh, odeg_oh = [], [], [], []
            for g in range(n_groups):
                zi = const.tile([P, NODE_DIM], BF16, tag=f"zin{g}")
                nc.gpsimd.dma_start(out=zi[:], in_=zin_d[g * P:(g + 1) * P, :])
                zin_sb.append(zi)
                zo = const.tile([P, NODE_DIM], BF16, tag=f"zout{g}")
                nc.gpsimd.dma_start(out=zo[:], in_=zout_d[g * P:(g + 1) * P, :])
                zout_sb.append(zo)
                for name, srcp, acc in (("i", idegr_d, ideg_oh),
                                        ("o", odegr_d, odeg_oh)):
                    db = const.tile([P, NPAD], BF16, tag=f"degb_{name}{g}")
                    nc.gpsimd.dma_start(
                        out=db[:],
                        in_=srcp[g:g + 1, :].to_broadcast([P, NPAD]),
                    )
                    oh = const.tile([P, NPAD], BF16, tag=f"oh_{name}{g}")
                    nc.vector.tensor_tensor(out=oh[:], in0=iotap_b[:],
                                            in1=db[:],
                                            op=mybir.AluOpType.is_equal)
                    acc.append(oh)

            def bcast_row(param, tag):
                t = const.tile([P, NODE_DIM], F32, tag=tag)
                nc.sync.dma_start(
                    out=t[:], in_=param[:].to_broadcast([P, NODE_DIM])
                )
                return t

            wc_b = bcast_row(wcent_d, "wc_b")
            wv_b = bcast_row(wvor_d, "wv_b")
            bs_b = bcast_row(bsum_d, "bs_b") if has_bsum else None
            bc_b = bcast_row(bcent_d, "bc_b") if has_bcent else None
            bv_b = bcast_row(bvor_d, "bv_b") if has_bvor else None

            cent_sb = const.tile([P, NPAIR], F32, tag="cent_sb")
            nc.sync.dma_start(out=cent_sb[:], in_=cent_d[:])
            vor_sb = const.tile([P, NPAIR], F32, tag="vor_sb")
            nc.sync.dma_start(out=vor_sb[:], in_=vor_d[:])
            if has_bsum:
                odegraw_sb = const.tile([P, NPAIR], F32, tag="odegraw_sb")
                nc.sync.dma_start(out=odegraw_sb[:], in_=odegraw_d[:])

            # --- block loads/stores -------------------------------------
            a_tiles: dict[int, object] = {}
            r_tiles: dict[int, object] = {}
            x_tiles: dict[int, object] = {}
            o_tiles: dict[int, object] = {}
            RW = TPG * 8  # srel_r8 cols per group

            def a_load(b):
                if not (0 <= b < NAB):
                    return
                nb = min(AB, NPAIR - b * AB)
                at = apool.tile([P, AB * GW], FP8)
                a_tiles[b] = at
                nc.sync.dma_start(
                    out=at[:, :nb * GW].rearrange("p (b c) -> p b c", b=nb),
                    in_=a_d[b * AB * P:(b * AB + nb) * P, :].rearrange(
                        "(b p) c -> p b c", p=P),
                )
                rt = rpool.tile([P, AB * RW], BF16, name="r_blk")
                r_tiles[b] = rt
                nc.sync.dma_start(
                    out=rt[:, :nb * RW],
                    in_=srel_d[:, b * AB * RW:(b * AB + nb) * RW],
                )

            def x_load(b):
                if not (0 <= b < NXB):
                    return
                nb = min(XB, NPAIR - b * XB)
                xt = xpool.tile([P, XB * NODE_DIM], FP8)
                x_tiles[b] = xt
                nc.sync.dma_start(
                    out=xt[:, :nb * NODE_DIM].rearrange("p (b c) -> p b c",
                                                        b=nb),
                    in_=x_d[b * XB * P:(b * XB + nb) * P, :].rearrange(
                        "(b p) c -> p b c", p=P),
                )

            # --- per-group phases ---------------------------------------
            sel_tiles: dict[int, object] = {}
            ps_tiles: dict[int, object] = {}
            agg_tiles: dict[int, object] = {}
            sv_tiles: dict[int, object] = {}

            def sel_build(cp):
                # sel[p, tt*64 + q] = (srel[p, tt] == q), tile-major so the
                # matmul rhs slices are contiguous (strided rhs is 3x slower)
                st = spool.tile([P, TPG * NW], BF16)
                sel_tiles[cp] = st
                rt = r_tiles[cp // AB]
                sl = rt[:, (cp % AB) * RW:(cp % AB + 1) * RW]
                nc.vector.tensor_tensor(
                    out=st[:].rearrange("p (t r j) -> p t r j", r=8, j=8),
                    in0=sl.rearrange("p (t j) -> p t j", j=8)[:, :, None, :]
                        .to_broadcast([P, TPG, 8, 8]),
                    in1=iota_q[:].rearrange("p (t r j) -> p t r j", r=8, j=8),
                    op=mybir.AluOpType.is_equal,
                )

            def phase1(cp):
                at = a_tiles[cp // AB]
                st = sel_tiles.pop(cp)
                boff = (cp % AB) * GW
                ps = psp.tile([P, P], F32, space="PSUM")
                ps_tiles[cp] = ps
                for tt in range(TPG):
                    s, t = tt // t_cap, tt % t_cap
                    nc.tensor.matmul(
                        out=ps[:, s * NW:(s + 1) * NW],
                        lhsT=at[:, boff + tt * P:boff + (tt + 1) * P],
                        rhs=st[:, tt * NW:(tt + 1) * NW],
                        start=(t == 0),
                        stop=(t == t_cap - 1),
                    )

            def phase1_cast(cp):
                # PSUM f32 -> SBUF bf16 on ACT (keeps DVE for sel builds)
                aggt = aggp.tile([P, P], BF16)
                agg_tiles[cp] = aggt
                nc.scalar.activation(out=aggt[:], in_=ps_tiles.pop(cp)[:],
                                     func=cpy)

            def phase2(cp):
                aggt = agg_tiles.pop(cp)
                xt = x_tiles[cp // XB]
                nsl = slice(cp * P, (cp + 1) * P)
                pp = prp.tile([P, NODE_DIM], F32, space="PSUM")
                nc.tensor.matmul(out=pp[:], lhsT=aggt[:], rhs=wsum_b[:],
                                 start=True, stop=False, skip_group_check=True)
                for g in range(n_groups):
                    nc.tensor.matmul(out=pp[:], lhsT=ideg_oh[g][:, nsl],
                                     rhs=zin_sb[g][:], start=False, stop=False,
                                     skip_group_check=True)
                    nc.tensor.matmul(out=pp[:], lhsT=odeg_oh[g][:, nsl],
                                     rhs=zout_sb[g][:], start=False, stop=False,
                                     skip_group_check=True)
                xoff = (cp % XB) * NODE_DIM
                nc.tensor.matmul(out=pp[:], lhsT=ident[:],
                                 rhs=xt[:, xoff:xoff + NODE_DIM],
                                 start=False, stop=True, skip_group_check=True)

                ct = tp.tile([P, NODE_DIM], BF16, tag="ct")
                if has_bcent:
                    ctf = tp.tile([P, NODE_DIM], F32, tag="ctf")
                    nc.vector.tensor_mul(
                        ctf[:], wc_b[:],
                        cent_sb[:, cp:cp + 1].to_broadcast([P, NODE_DIM]))
                    nc.vector.tensor_add(ctf[:], ctf[:], bc_b[:])
                    nc.scalar.activation(out=ct[:], in_=ctf[:], func=sig)
                else:
                    nc.scalar.activation(out=ct[:], in_=wc_b[:], func=sig,
                                         scale=cent_sb[:, cp:cp + 1])
                vt = tp.tile([P, NODE_DIM], BF16, tag="vt")
                if has_bvor:
                    vtf = tp.tile([P, NODE_DIM], F32, tag="vtf")
                    nc.vector.tensor_mul(
                        vtf[:], wv_b[:],
                        vor_sb[:, cp:cp + 1].to_broadcast([P, NODE_DIM]))
                    nc.vector.tensor_add(vtf[:], vtf[:], bv_b[:])
                    nc.scalar.activation(out=vt[:], in_=vtf[:], func=sig)
                else:
                    nc.scalar.activation(out=vt[:], in_=wv_b[:], func=sig,
                                         scale=vor_sb[:, cp:cp + 1])

                sv = tp.tile([P, NODE_DIM], BF16, tag="sv")
                nc.gpsimd.tensor_add(sv[:], ct[:], vt[:])
                if has_bsum:
                    bst = tp.tile([P, NODE_DIM], F32, tag="bst")
                    nc.vector.tensor_mul(
                        bst[:], bs_b[:],
                        odegraw_sb[:, cp:cp + 1].to_broadcast([P, NODE_DIM]))
                    nc.vector.tensor_add(bst[:], bst[:], sv[:])
                    sv = bst
                sv_tiles[cp] = (sv, pp)

            def phase2_fin(cp):
                # DVE add, emitted after sel_build(cp+LAG) so the serial
                # chain p2 -> o-add -> sel -> p1 is broken
                sv, pp = sv_tiles.pop(cp)
                ob, oi = cp // AB, cp % AB
                if oi == 0:
                    o_tiles[ob] = opool.tile([P, AB * NODE_DIM], BF16,
                                             name="o_blk")
                o = o_tiles[ob]
                nc.vector.tensor_add(
                    o[:, oi * NODE_DIM:(oi + 1) * NODE_DIM], sv[:], pp[:])

            def store(b):
                nb = min(AB, NPAIR - b * AB)
                o = o_tiles.pop(b)
                nc.scalar.dma_start(
                    out=out_d[b * AB * P:(b * AB + nb) * P, :].rearrange(
                        "(b p) c -> p b c", p=P),
                    in_=o[:, :nb * NODE_DIM].rearrange("p (b c) -> p b c",
                                                       b=nb),
                )

            # --- pipeline -----------------------------------------------
            CAST_LAG, LAG, SLAG = 2, 6, 10
            a_load(0)
            x_load(0)
            for cp in range(NPAIR + SLAG):
                if LAG <= cp < NPAIR + LAG:
                    phase2(cp - LAG)
                if cp >= SLAG:
                    g = cp - SLAG
                    if g % AB == AB - 1 or g == NPAIR - 1:
                        store(g // AB)
                if cp < NPAIR:
                    if cp % AB == 0:
                        a_load(cp // AB + 1)
                    if cp % XB == 0:
                        x_load(cp // XB + 1)
                    sel_build(cp)
                    phase1(cp)
                if LAG <= cp < NPAIR + LAG:
                    phase2_fin(cp - LAG)
                if CAST_LAG <= cp < NPAIR + CAST_LAG:
                    phase1_cast(cp - CAST_LAG)

    return nc


# ----------------------------------------------------------------------------
def _pack_nodes(out_deg):
    """LPT bin-packing: nodes -> (core, slot) balancing per-core and
    per-sub-chunk edge counts.  Returns (perm[N_CORES, NPAD] node-or--1,
    t_cap)."""
    order = np.argsort(-out_deg, kind="stable")
    # cores (count cap NPC)
    core_of = np.empty(N_NODES, np.int32)
    csum = np.zeros(N_CORES, np.int64)
    ccnt = np.zeros(N_CORES, np.int64)
    heap = [(0, c) for c in range(N_CORES)]
    heapq.heapify(heap)
    for n in order:
        while True:
            _, c = heapq.heappop(heap)
            if ccnt[c] < NPC:
                break
        core_of[n] = c
        ccnt[c] += 1
        csum[c] += out_deg[n]
        if ccnt[c] < NPC:
            heapq.heappush(heap, (csum[c], c))
    # bins within each core (count cap NW)
    perm = np.full((N_CORES, NPAD), -1, np.int64)
    slot_of = np.empty(N_NODES, np.int64)
    worst = 0
    for c in range(N_CORES):
        nodes = order[core_of[order] == c]
        bsum = np.zeros(CB, np.int64)
        bcnt = np.zeros(CB, np.int64)
        bh = [(0, b) for b in range(CB)]
        heapq.heapify(bh)
        for n in nodes:
            while True:
                _, b = heapq.heappop(bh)
                if bcnt[b] < NW:
                    break
            s = b * NW + bcnt[b]
            perm[c, s] = n
            slot_of[n] = s
            bcnt[b] += 1
            bsum[b] += out_deg[n]
            if bcnt[b] < NW:
                heapq.heappush(bh, (bsum[b], b))
        worst = max(worst, int(bsum.max()))
    t_cap = max((worst + P - 1) // P, 1)
    return perm, core_of, slot_of, t_cap


def prepare_inputs(x, edge_index, edge_attr, voronoi_values, centralities,
                   z_in, z_out, W_cent, b_cent, W_vor, b_vor, W_sum, b_sum):
    """Host-side sharding: LPT-pack nodes into (core, sub-chunk) slots,
    route each edge to its src slot, pad sub-chunks to t_cap*128 edges.
    Returns (in_maps, build_key, perm)."""
    src = np.asarray(edge_index[0], dtype=np.int64)
    dst = np.asarray(edge_index[1], dtype=np.int64)
    edge_attr = np.asarray(edge_attr, dtype=np.float32)
    x = np.asarray(x, dtype=np.float32)

    out_deg_raw = np.bincount(src, minlength=N_NODES).astype(np.int64)
    in_deg_raw = np.bincount(dst, minlength=N_NODES).astype(np.int64)
    in_deg = np.minimum(in_deg_raw, MAX_DEG - 1).astype(np.int64)
    out_deg = np.minimum(out_deg_raw, MAX_DEG - 1).astype(np.int64)
    n_groups = max(int(max(in_deg.max(), out_deg.max())) // P + 1, 1)

    perm, core_of_node, slot_of_node, t_cap = _pack_nodes(out_deg_raw)
    TPG = SPG * t_cap
    GW = TPG * P

    core_of = core_of_node[src]
    slot = slot_of_node[src]
    gchunk = core_of.astype(np.int64) * CB + slot // NW
    srel = (slot % NW).astype(np.float32)

    order = np.argsort(gchunk, kind="stable")
    gchunk_s = gchunk[order]
    n_chunks = N_CORES * CB
    counts = np.bincount(gchunk_s, minlength=n_chunks)
    assert counts.max() <= t_cap * P
    starts = np.zeros(n_chunks, dtype=np.int64)
    starts[1:] = np.cumsum(counts)[:-1]
    eslot = np.arange(N_EDGES, dtype=np.int64) - starts[gchunk_s]
    # a: flat row in the [group, P(partition)] grid; col = (tile, d)
    ggrp = gchunk_s // SPG
    sub = gchunk_s % SPG
    tt = sub * t_cap + eslot // P
    pos_a = (ggrp * P + eslot % P) * TPG + tt
    pos_s = (gchunk_s * P + eslot % P) * t_cap + eslot // P

    a_grid = np.zeros((N_CORES * NPAIR * P * TPG, EDGE_DIM),
                      dtype=ml_dtypes.float8_e3m4)
    a_grid[pos_a] = edge_attr[order].astype(ml_dtypes.float8_e3m4)
    a_grid = a_grid.reshape(N_CORES, NPAIR * P, GW)
    srel_grid = np.full(n_chunks * P * t_cap, -1.0, dtype=ml_dtypes.bfloat16)
    srel_grid[pos_s] = srel[order].astype(ml_dtypes.bfloat16)
    # repeat each srel value 8x (dense runs for the DVE sel build)
    srel_grid = np.repeat(srel_grid.reshape(N_CORES, CB, P, t_cap), 8,
                          axis=-1)

    def gather_nodes(arr, fill=0):
        # [N_NODES, ...] -> [N_CORES, NPAD, ...] via perm
        shp = (N_CORES, NPAD) + arr.shape[1:]
        outa = np.full(shp, fill, dtype=arr.dtype)
        m = perm >= 0
        outa[m] = arr[perm[m]]
        return outa

    x_p = gather_nodes(x)
    ideg_p = gather_nodes(in_deg.astype(np.float32)[:, None],
                          fill=65536.0)[..., 0]
    odeg_p = gather_nodes(out_deg.astype(np.float32)[:, None],
                          fill=65536.0)[..., 0]
    goff = (np.arange(n_groups, dtype=np.float32) * P)[None, :, None]
    ideg_rows = np.ascontiguousarray(ideg_p[:, None, :] - goff)
    odeg_rows = np.ascontiguousarray(odeg_p[:, None, :] - goff)
    cent_p = gather_nodes(np.asarray(centralities, dtype=np.float32))
    vor_p = gather_nodes(np.asarray(voronoi_values, dtype=np.float32))

    def col_layout(a):  # [NPAD,1] -> [P, NPAIR]
        return np.ascontiguousarray(a.reshape(NPAIR, P).T)

    bf = lambda v: np.ascontiguousarray(np.asarray(v).astype(ml_dtypes.bfloat16))
    row = lambda v: np.ascontiguousarray(
        np.asarray(v, dtype=np.float32).reshape(1, NODE_DIM))
    z_in_b = bf(np.asarray(z_in, np.float32)[:n_groups * P])
    z_out_b = bf(np.asarray(z_out, np.float32)[:n_groups * P])
    b_sum_r, b_cent_r, b_vor_r = row(b_sum), row(b_cent), row(b_vor)
    flags = (bool(np.any(b_sum_r)), bool(np.any(b_cent_r)),
             bool(np.any(b_vor_r)))

    in_maps = []
    for c in range(N_CORES):
        m = {
            "a": np.ascontiguousarray(a_grid[c]),
            "srel": np.ascontiguousarray(
                srel_grid[c].transpose(1, 0, 2).reshape(P, CB * t_cap * 8)),
            "x": np.ascontiguousarray(x_p[c].astype(ml_dtypes.float8_e3m4)),
            "idegrow": ideg_rows[c],
            "odegrow": odeg_rows[c],
            "cent": col_layout(cent_p[c]),
            "vor": col_layout(vor_p[c]),
            "z_in": z_in_b,
            "z_out": z_out_b,
            "W_sum": bf(np.asarray(W_sum, np.float32)),
            "W_cent": row(W_cent),
            "W_vor": row(W_vor),
        }
        if flags[0]:
            m["odegraw"] = col_layout(
                gather_nodes(out_deg_raw.astype(np.float32)[:, None])[c])
            m["b_sum"] = b_sum_r
        if flags[1]:
            m["b_cent"] = b_cent_r
        if flags[2]:
            m["b_vor"] = b_vor_r
        in_maps.append(m)
    return in_maps, (t_cap, n_groups) + flags, perm


_PROGRAM_CACHE: dict[tuple, bass.Bass] = {}


def kernel(**inputs) -> np.ndarray:
    in_maps, key, perm = prepare_inputs(**inputs)
    nc = _PROGRAM_CACHE.get(key)
    if nc is None:
        nc = build_program(*key)
        _PROGRAM_CACHE[key] = nc
    res = None
    for attempt in range(3):
        try:
            res = run_bass_kernel_spmd(nc, in_maps, core_ids=list(range(N_CORES)))
            break
        except Exception:
            # axon transiently reports "accelerator device unrecoverable";
            # a clean retry succeeds
            if attempt == 2:
                raise
    out = np.empty((N_NODES, NODE_DIM), np.float32)
    for c in range(N_CORES):
        r = np.asarray(res.results[c]["out"]).astype(np.float32)
        m = perm[c] >= 0
        out[perm[c][m]] = r[m]
    return out
